# revision 56
# baseline (speedup 1.0000x reference)
"""Multi-head attention kernel for Trainium2, 8 NeuronCores.

Problem: B=4, T=2048, D=1024, H=16 heads (Hd=64), fp32, full softmax
attention with key-padding mask + output projection.

Sharding: batch x head-half.  Core c handles batch c//2 and heads
8*(c%2)..8*(c%2)+7 (feature slice of 512).  Each core computes a partial
output projection (Wo row-sharded); the host sums the two partials per
batch and adds bo.

Device-side strategy (all matmuls bf16 at full PE rate; exp is the
attention pacer and is split across ScalarE and VectorE):
  - x is transposed on host -> xT [D, T]; Q^T, K^T computed in [feat, T]
    layout so S^T = K^T.T @ Q^T has keys on partitions; head pairs share
    one 128-partition tile so the two 64-contraction S^T matmuls run
    concurrently via PE row tiling.  The softmax 1/sqrt(Hd) is applied
    at the exp stage (ACT scale / Schraudolph slope), not in Wq.
  - One x-streaming pass computes Q, K (bf16) and V; V rows are scaled
    by keep=1-mask and carry a 65th keep column so the PV matmul yields
    softmax denominators for free (no max-subtraction needed for these
    input stats).
  - The whole attention is ONE flat software-pipelined loop over
    (j, hp, c): score matmuls run 2 iterations ahead of exp; exp(c)
    alternates between ScalarE (table exp, 10/16) and VectorE
    (bf16 Schraudolph via int16 writeback, 6/16); PV consumes pt
    per half-tile.  Per-query-block normalization (fast-reciprocal of
    the keep row + GPSIMD broadcast + multiply) is deferred into the
    next block so its latency hides behind the stream, and the output
    projection is emitted as 2-matmul-group pieces spread across the
    following query block, with psum-buffer parity guarded so pieces
    never collide with the live PV accumulators.
  - ACT/DVE/GPSIMD one-time costs (exp table, custom-op config) are
    warmed up during pass 0.
"""
import sys
sys.path.insert(0, "/opt/trn_rl_repo")

from contextlib import ExitStack

import numpy as np
import ml_dtypes
import concourse.bass as bass
import concourse.mybir as mybir
import concourse.tile as tile
from concourse import bacc
from concourse.bass_utils import run_bass_kernel_spmd

B, T, D, H = 4, 2048, 1024, 16
Hd = D // H          # 64
HH = H // 2          # 8 heads per core
FH = HH * Hd         # 512 features per core
P = 128
NCHUNK = T // 512    # 4 query/T chunks
NDC = D // P         # 8 contraction chunks for projections
NKT = T // P         # 16 key tiles
NFT = FH // P        # 4 feature tiles per core

f32 = mybir.dt.float32
r32 = mybir.dt.float32r
i16 = mybir.dt.int16
SCALE = 0.125  # 1/sqrt(Hd), applied at the exp stage
ADD = mybir.AluOpType.add
MULT = mybir.AluOpType.mult
EXP = mybir.ActivationFunctionType.Exp

# Schraudolph fast-exp in bf16: bf16 bits = top 16 of f32 bits, so
# int16(x*A/2^16 + B/2^16) bit-viewed as bf16 ~= exp(x).  The constant
# multiplicative bias of the approximation cancels in softmax num/den; only
# the ~2% rms sawtooth survives, attenuated by 1/sqrt(N_eff) in the output.
EXP_A = 12102203.161561485 / 65536.0 * SCALE   # 2**23/ln2/2**16 * scale
EXP_B = 1064866805.0 / 65536.0
# c-chunks whose exp runs on DVE (Schraudolph) instead of ACT
DVE_EXP_SET = (5, 7, 9, 11, 13, 15)

_cache = {}


def _round_fp32r(a):
    """Round fp32 array to fp32r (11 mantissa bits, round-nearest-even)."""
    b = np.ascontiguousarray(a, dtype=np.float32).view(np.uint32).astype(np.uint64)
    drop = 12
    half = np.uint64(1 << (drop - 1))
    lsb = (b >> np.uint64(drop)) & np.uint64(1)
    keepmask = np.uint64(~((1 << drop) - 1) & 0xFFFFFFFF)
    r = (b + half - np.uint64(1) + lsb) & keepmask
    return r.astype(np.uint32).view(np.float32).reshape(np.shape(a))


def _build():
    nc = bacc.Bacc(None, target_bir_lowering=False)
    bf16 = mybir.dt.bfloat16
    # packed layouts: per-partition lines are long and DRAM-contiguous
    xh0 = nc.declare_dram_parameter("xh0", [P, NDC * 1024], bf16, isOutput=False)
    xh1 = nc.declare_dram_parameter("xh1", [P, NDC * 1024], bf16, isOutput=False)
    wq = nc.declare_dram_parameter("wq", [P, NDC * FH], bf16, isOutput=False)
    wk = nc.declare_dram_parameter("wk", [P, NDC * FH], bf16, isOutput=False)
    wv = nc.declare_dram_parameter("wv", [P, NDC * FH], bf16, isOutput=False)
    wo = nc.declare_dram_parameter("wo", [P, NFT * D], bf16, isOutput=False)
    bq = nc.declare_dram_parameter("bq", [FH], f32, isOutput=False)
    bk = nc.declare_dram_parameter("bk", [FH], f32, isOutput=False)
    bvr = nc.declare_dram_parameter("bvr", [P, FH], f32, isOutput=False)
    keep = nc.declare_dram_parameter("keep", [T], r32, isOutput=False)
    outT = nc.declare_dram_parameter("outT", [D, T], f32, isOutput=True)
    xh = [xh0, xh1]

    with tile.TileContext(nc) as tc, ExitStack() as ctx:
        const = ctx.enter_context(tc.tile_pool(name="const", bufs=1))
        qt_pool = ctx.enter_context(tc.tile_pool(name="qt", bufs=1))
        kt_pool = ctx.enter_context(tc.tile_pool(name="kt", bufs=1))
        v_pool = ctx.enter_context(tc.tile_pool(name="v", bufs=1))
        o_pool = ctx.enter_context(tc.tile_pool(name="o", bufs=1))
        ps = ctx.enter_context(tc.tile_pool(name="ps", bufs=1, space="PSUM"))
        x1_pool = ctx.enter_context(tc.tile_pool(name="x1", bufs=2))
        w_pool = ctx.enter_context(tc.tile_pool(name="w", bufs=1))

        # engine warmups: pay ACT exp-table load, custom-DVE table config,
        # and GPSIMD launch costs during pass 0 instead of attention start
        warm = const.tile([1, 16], f32, tag="warm")
        nc.vector.memset(warm, 1.0)
        warm2 = const.tile([1, 16], f32, tag="warm2")
        nc.scalar.activation(warm2, warm, EXP)
        warm3 = const.tile([1, 16], f32, tag="warm3")
        nc.vector.reciprocal_approx_fast(warm3, warm)
        warmb = const.tile([Hd, 16], f32, tag="warmb")
        nc.gpsimd.partition_broadcast(warmb, warm)

        # constants / biases
        bq_sb = const.tile([P, NFT], f32, tag="bq")
        bk_sb = const.tile([P, NFT], f32, tag="bk")
        nc.sync.dma_start(out=bq_sb, in_=bq.rearrange("(f p) -> p f", p=P))
        nc.sync.dma_start(out=bk_sb, in_=bk.rearrange("(f p) -> p f", p=P))
        keep_sb = const.tile([P, NKT], r32, tag="keep")
        nc.sync.dma_start(out=keep_sb, in_=keep.rearrange("(c p) -> p c", p=P))
        zeros8 = const.tile([P, HH], f32, tag="zeros8")
        nc.vector.memset(zeros8, 0.0)

        # persistent activations (bf16: fp8 is too noisy for the 2e-2
        # tolerance -- softmax weight noise passes straight to rel err)
        QT = [qt_pool.tile([P, T], bf16, tag=f"qt{i}", name=f"qt{i}")
              for i in range(NFT)]
        KT = [kt_pool.tile([P, T], bf16, tag=f"kt{i}", name=f"kt{i}")
              for i in range(NFT)]
        V = [v_pool.tile([P, HH, Hd + 1], bf16, tag=f"v{i}", name=f"v{i}")
             for i in range(NKT)]
        O = [o_pool.tile([P, T], bf16, tag=f"o{i}", name=f"o{i}")
             for i in range(NFT)]

        # Q/K weights: one packed tile each, [128, dc, f]; DMAs are issued
        # inside pass 0, interleaved with the x chunks
        wq_b = w_pool.tile([P, NDC, FH], bf16, tag="wqb", name="wq_b")
        wk_b = w_pool.tile([P, NDC, FH], bf16, tag="wkb", name="wk_b")

        def psum_wide(name):
            return ps.tile([P, 1024], f32, tag="st", bufs=2, name=name)

        def psum_pv(name, which):
            return ps.tile([P, 512], f32, tag=("pva" if which == 0 else "pvb"),
                           bufs=2, name=name)

        def qk_psum(f, n, xb, off):
            # xb: [P, dc, 1024] packed half tile; off: column offset in half
            ts = slice(n * 512, (n + 1) * 512)
            fs = slice(f * P, (f + 1) * P)
            pqk = psum_wide("pqk")
            psq = pqk[:, 0:512]
            for dc in range(NDC):
                nc.tensor.matmul(psq, wq_b[:, dc, fs],
                                 xb[:, dc, off:off + 512],
                                 start=(dc == 0), stop=(dc == NDC - 1))
            nc.vector.tensor_scalar_add(
                QT[f][:, ts], psq, bq_sb[:, f:f + 1])
            psk = pqk[:, 512:1024]
            for dc in range(NDC):
                nc.tensor.matmul(psk, wk_b[:, dc, fs],
                                 xb[:, dc, off:off + 512],
                                 start=(dc == 0), stop=(dc == NDC - 1))
            nc.vector.tensor_scalar_add(
                KT[f][:, ts], psk, bk_sb[:, f:f + 1])

        # ------- pass 0: V projection + Q/K projection ----------------
        with nc.named_scope("v_qk0"), ExitStack() as p0:
            wv_pool = p0.enter_context(tc.tile_pool(name="wv", bufs=1))
            vt_pool = p0.enter_context(tc.tile_pool(name="vt", bufs=2))
            xbs = [x1_pool.tile([P, NDC, 1024], bf16, tag="xh", name="xb")
                   for _ in range(2)]
            for i in range(4):
                cs2 = slice(i * 2 * FH, (i + 1) * 2 * FH)
                for k in range(2):
                    dc = 2 * i + k
                    nc.sync.dma_start(
                        out=xbs[0][:, dc, :],
                        in_=xh[0][:, dc * 1024:(dc + 1) * 1024])
                nc.sync.dma_start(out=wq_b[:, 2 * i:2 * i + 2, :],
                                  in_=wq[:, cs2])
                nc.sync.dma_start(out=wk_b[:, 2 * i:2 * i + 2, :],
                                  in_=wk[:, cs2])
            bvr_sb = vt_pool.tile([P, FH], f32, tag="bvr", bufs=1,
                                  name="bvr_sb")
            nc.sync.dma_start(out=bvr_sb, in_=bvr[:])
            wv_b = wv_pool.tile([P, NDC, FH], bf16, tag="wvb", name="wv_b")
            for i in range(4):
                cs2 = slice(i * 2 * FH, (i + 1) * 2 * FH)
                nc.sync.dma_start(out=wv_b[:, 2 * i:2 * i + 2, :],
                                  in_=wv[:, cs2])
            for dc in range(NDC):
                nc.sync.dma_start(
                    out=xbs[1][:, dc, :],
                    in_=xh[1][:, dc * 1024:(dc + 1) * 1024])
            for nh in range(2):
                xb = xbs[nh]
                for f_ in range(2):
                    for nn in range(2):
                        qk_psum(f_, nh * 2 + nn, xb, nn * 512)
                for s in range(8):
                    tidx = nh * 8 + s
                    ss = slice(s * P, (s + 1) * P)
                    psv = psum_pv("psv", s % 2)
                    for dc in range(NDC):
                        nc.tensor.matmul(psv, xb[:, dc, ss],
                                         wv_b[:, dc, :],
                                         start=(dc == 0),
                                         stop=(dc == NDC - 1))
                    vtmp = vt_pool.tile([P, FH], f32, tag="vtmp",
                                        name="vtmp")
                    nc.vector.tensor_tensor(vtmp, psv, bvr_sb,
                                            op=ADD)
                    nc.vector.tensor_scalar_mul(
                        V[tidx][:, :, 0:Hd],
                        vtmp.rearrange("p (h d) -> p h d", h=HH),
                        keep_sb[:, tidx:tidx + 1].bitcast(f32))
                    nc.vector.tensor_scalar_add(
                        V[tidx][:, :, Hd], zeros8,
                        keep_sb[:, tidx:tidx + 1].bitcast(f32))

        # ------- attention + deferred Q/K tiles + projection ----------
        with ExitStack() as pw:
            pt_pool = pw.enter_context(tc.tile_pool(name="pt", bufs=6))
            rc_pool = pw.enter_context(tc.tile_pool(name="rc", bufs=2))
            wo_pool = pw.enter_context(tc.tile_pool(name="wo", bufs=1))
            ot_pool = pw.enter_context(tc.tile_pool(name="ot", bufs=6))

            wo_b = wo_pool.tile([P, NFT, D], bf16, tag="wob", name="wo_b")
            for i in range(2):
                nc.sync.dma_start(out=wo_b[:, 2 * i:2 * i + 2, :],
                                  in_=wo[:, i * 2 * D:(i + 1) * 2 * D])

            alloc_cnt = {0: 0, 1: 0}
            live_slot = {0: -1, 1: -1}

            def pv_alloc(which):
                t = psum_pv("pv", which)
                live_slot[which] = alloc_cnt[which] % 2
                alloc_cnt[which] += 1
                return t

            def pso_alloc(which):
                if alloc_cnt[which] % 2 == live_slot[which]:
                    psum_pv("dummy", which)   # parity shift, no readers
                    alloc_cnt[which] += 1
                t = psum_pv("pso", which)
                alloc_cnt[which] += 1
                return t

            def proj_piece(j, dt_):
                js = slice(j * 512, (j + 1) * 512)
                ds_ = slice(dt_ * P, (dt_ + 1) * P)
                pso = pso_alloc(dt_ % 2)
                for fc in range(NFT):
                    nc.tensor.matmul(pso,
                                     wo_b[:, fc, ds_],
                                     O[fc][:, js],
                                     start=(fc == 0),
                                     stop=(fc == NFT - 1))
                ot = ot_pool.tile([P, 512], f32, tag="ot", name="ot")
                nc.scalar.copy(ot, pso)
                nc.sync.dma_start(out=outT[ds_, js], in_=ot)

            def proj_j(j):
                for dt_ in range(NDC):
                    proj_piece(j, dt_)

            def normalize(hp, j, pvA, pvB):
                js = slice(j * 512, (j + 1) * 512)
                for h, pv in ((0, pvA), (1, pvB)):
                    den = rc_pool.tile([1, 512], f32, tag="den",
                                       bufs=2, name="den")
                    nc.vector.tensor_copy(den, pv[Hd:Hd + 1, :])
                    rec = rc_pool.tile([1, 512], f32, tag="rec",
                                       bufs=2, name="rec")
                    nc.vector.reciprocal_approx_fast(rec, den)
                    rrep = rc_pool.tile([Hd, 512], f32, tag="rrep",
                                        bufs=2, name="rrep")
                    nc.gpsimd.partition_broadcast(rrep, rec)
                    rows = slice(h * Hd, (h + 1) * Hd)
                    nc.vector.tensor_tensor(
                        O[hp][rows, js], pv[0:Hd, :], rrep, op=MULT)

            NT = NFT * NCHUNK * NKT
            sts = {}
            pvs = {}

            def idx(t):
                j, r = divmod(t, NFT * NKT)
                hp, c = divmod(r, NKT)
                return hp, j, c

            def scores(t):
                hp, j, c = idx(t)
                js = slice(j * 512, (j + 1) * 512)
                cs = slice(c * P, (c + 1) * P)
                st = psum_wide("st")
                nc.tensor.matmul(st[:, 0:512],
                                 KT[hp][0:64, cs],
                                 QT[hp][0:64, js],
                                 start=True, stop=True,
                                 tile_position=(0, 0))
                nc.tensor.matmul(st[:, 512:1024],
                                 KT[hp][64:128, cs],
                                 QT[hp][64:128, js],
                                 start=True, stop=True,
                                 tile_position=(64, 0))
                sts[t] = st

            def norm_head(hp, j, h, pv):
                js = slice(j * 512, (j + 1) * 512)
                den = rc_pool.tile([1, 512], f32, tag="den",
                                   bufs=2, name="den")
                nc.vector.tensor_copy(den, pv[Hd:Hd + 1, :])
                rec = rc_pool.tile([1, 512], f32, tag="rec",
                                   bufs=2, name="rec")
                nc.vector.reciprocal_approx_fast(rec, den)
                rrep = rc_pool.tile([Hd, 512], f32, tag="rrep",
                                    bufs=2, name="rrep")
                nc.gpsimd.partition_broadcast(rrep, rec)
                rows = slice(h * Hd, (h + 1) * Hd)
                nc.vector.tensor_tensor(
                    O[hp][rows, js], pv[0:Hd, :], rrep, op=MULT)

            def qk_piece(f, n):
                # deferred Q/K projection for one (f, T-chunk): two
                # parity-guarded [P,512] psum pieces on the pv tags
                ts = slice(n * 512, (n + 1) * 512)
                fs = slice(f * P, (f + 1) * P)
                xb = xbs[n // 2]
                off = (n % 2) * 512
                for which, wb, dst, bias in ((0, wq_b, QT, bq_sb),
                                             (1, wk_b, KT, bk_sb)):
                    pp = pso_alloc(which)
                    for dc in range(NDC):
                        nc.tensor.matmul(pp, wb[:, dc, fs],
                                         xb[:, dc, off:off + 512],
                                         start=(dc == 0),
                                         stop=(dc == NDC - 1))
                    nc.vector.tensor_scalar_add(
                        dst[f][:, ts], pp, bias[:, f:f + 1])

            def attn():
                scores(0)
                scores(1)
                pending = []
                proj_pieces = []
                qk_deferred = [(f, n) for f in (2, 3) for n in range(4)]
                for t in range(NT):
                    hp, j, c = idx(t)
                    js = slice(j * 512, (j + 1) * 512)
                    if c == 0:
                        pvs[0] = (pv_alloc(0), pv_alloc(1))
                    pvA, pvB = pvs[0]
                    if t + 2 < NT:
                        scores(t + 2)
                    st = sts.pop(t)
                    pt = pt_pool.tile([P, 1024], bf16, tag="pt",
                                      name="pt")
                    if c in DVE_EXP_SET:
                        nc.vector.tensor_scalar(
                            out=pt.bitcast(i16), in0=st,
                            scalar1=EXP_A, scalar2=EXP_B,
                            op0=MULT, op1=ADD)
                    else:
                        nc.scalar.activation(pt, st, EXP, scale=SCALE)
                    if c == 2 and pending:
                        php, pj, ppvA, ppvB = pending[0]
                        norm_head(php, pj, 0, ppvA)
                    if c == 5 and pending:
                        php, pj, ppvA, ppvB = pending.pop(0)
                        norm_head(php, pj, 1, ppvB)
                        if php == NFT - 1:
                            proj_pieces.extend((pj, d) for d in range(NDC))
                    if c == 6:
                        for _ in range(2):
                            if proj_pieces:
                                pj2, d2 = proj_pieces.pop(0)
                                proj_piece(pj2, d2)
                    if c in (7, 10, 13) and qk_deferred:
                        qk_piece(*qk_deferred.pop(0))
                    nc.tensor.matmul(pvA[0:Hd + 1, :],
                                     V[c][:, 2 * hp, :],
                                     pt[:, 0:512],
                                     start=(c == 0),
                                     stop=(c == NKT - 1))
                    nc.tensor.matmul(pvB[0:Hd + 1, :],
                                     V[c][:, 2 * hp + 1, :],
                                     pt[:, 512:1024],
                                     start=(c == 0),
                                     stop=(c == NKT - 1))
                    if c == NKT - 1:
                        pending.append((hp, j, pvA, pvB))
                for php, pj, ppvA, ppvB in pending:
                    norm_head(php, pj, 0, ppvA)
                    norm_head(php, pj, 1, ppvB)
                    if php == NFT - 1:
                        proj_pieces.extend((pj, d) for d in range(NDC))
                pending.clear()
                for pj2, d2 in proj_pieces:
                    proj_piece(pj2, d2)
                proj_pieces.clear()

            with nc.named_scope("attn"):
                attn()

    nc.compile()
    return nc


def _get_nc():
    if "nc" not in _cache:
        _cache["nc"] = _build()
    return _cache["nc"]


def kernel(x, mask, Wq, bq, Wk, bk, Wv, bv, Wo, bo):
    x = np.asarray(x, dtype=np.float32)
    mask = np.asarray(mask)
    Wq = np.asarray(Wq, dtype=np.float32)
    bq = np.asarray(bq, dtype=np.float32)
    Wk = np.asarray(Wk, dtype=np.float32)
    bk = np.asarray(bk, dtype=np.float32)
    Wv = np.asarray(Wv, dtype=np.float32)
    bv = np.asarray(bv, dtype=np.float32)
    Wo = np.asarray(Wo, dtype=np.float32)
    bo = np.asarray(bo, dtype=np.float32)

    scale = np.float32(Hd) ** -0.5
    nc = _get_nc()

    def pack_w(w):
        # [D, FH] -> [128, (dc f)]: partition p line = concat over dc of
        # w[dc*128+p, :]
        return np.ascontiguousarray(
            w.astype(ml_dtypes.bfloat16).reshape(NDC, P, FH)
            .transpose(1, 0, 2).reshape(P, NDC * FH))

    in_maps = []
    for core in range(8):
        b, s = core // 2, core % 2
        sl = slice(s * FH, (s + 1) * FH)
        xr = x[b].T.astype(ml_dtypes.bfloat16).reshape(NDC, P, T)
        wo_p = (Wo[sl, :].astype(ml_dtypes.bfloat16)
                .reshape(NFT, P, D).transpose(1, 0, 2).reshape(P, NFT * D))
        m = {
            "xh0": np.ascontiguousarray(
                xr[:, :, 0:1024].transpose(1, 0, 2).reshape(P, NDC * 1024)),
            "xh1": np.ascontiguousarray(
                xr[:, :, 1024:2048].transpose(1, 0, 2).reshape(P, NDC * 1024)),
            "wq": pack_w(Wq[:, sl]),
            "wk": pack_w(Wk[:, sl]),
            "wv": pack_w(Wv[:, sl]),
            "wo": np.ascontiguousarray(wo_p),
            "bq": np.ascontiguousarray(bq[sl]),
            "bk": np.ascontiguousarray(bk[sl]),
            "bvr": np.ascontiguousarray(np.broadcast_to(bv[sl], (P, FH))),
            "keep": (1.0 - mask[b].astype(np.float32)),
        }
        in_maps.append(m)

    global _last_in_maps
    _last_in_maps = in_maps
    res = run_bass_kernel_spmd(nc, in_maps, list(range(8)))
    out = np.empty((B, T, D), dtype=np.float32)
    for b in range(B):
        acc = res.results[2 * b]["outT"] + res.results[2 * b + 1]["outT"]
        out[b] = acc.T + bo
    return out



# revision 57
# speedup vs baseline: 1.0149x; 1.0149x over previous
"""Multi-head attention kernel for Trainium2, 8 NeuronCores.

Problem: B=4, T=2048, D=1024, H=16 heads (Hd=64), fp32, full softmax
attention with key-padding mask + output projection.

Sharding: batch x head-half.  Core c handles batch c//2 and heads
8*(c%2)..8*(c%2)+7 (feature slice of 512).  Each core computes a partial
output projection (Wo row-sharded); the host sums the two partials per
batch and adds bo.

Device-side strategy (all matmuls bf16 at full PE rate; exp is the
attention pacer and is split across ScalarE and VectorE):
  - x is transposed on host -> xT [D, T]; Q^T, K^T computed in [feat, T]
    layout so S^T = K^T.T @ Q^T has keys on partitions; head pairs share
    one 128-partition tile so the two 64-contraction S^T matmuls run
    concurrently via PE row tiling.  The softmax 1/sqrt(Hd) is applied
    at the exp stage (ACT scale / Schraudolph slope), not in Wq.
  - One x-streaming pass computes Q, K (bf16) and V; V rows are scaled
    by keep=1-mask and carry a 65th keep column so the PV matmul yields
    softmax denominators for free (no max-subtraction needed for these
    input stats).
  - The whole attention is ONE flat software-pipelined loop over
    (j, hp, c): score matmuls run 2 iterations ahead of exp; exp(c)
    alternates between ScalarE (table exp, 10/16) and VectorE
    (bf16 Schraudolph via int16 writeback, 6/16); PV consumes pt
    per half-tile.  Per-query-block normalization (fast-reciprocal of
    the keep row + GPSIMD broadcast + multiply) is deferred into the
    next block so its latency hides behind the stream, and the output
    projection is emitted as 2-matmul-group pieces spread across the
    following query block, with psum-buffer parity guarded so pieces
    never collide with the live PV accumulators.
  - ACT/DVE/GPSIMD one-time costs (exp table, custom-op config) are
    warmed up during pass 0.
"""
import sys
sys.path.insert(0, "/opt/trn_rl_repo")

from contextlib import ExitStack

import numpy as np
import ml_dtypes
import concourse.bass as bass
import concourse.mybir as mybir
import concourse.tile as tile
from concourse import bacc
from concourse.bass_utils import run_bass_kernel_spmd

B, T, D, H = 4, 2048, 1024, 16
Hd = D // H          # 64
HH = H // 2          # 8 heads per core
FH = HH * Hd         # 512 features per core
P = 128
NCHUNK = T // 512    # 4 query/T chunks
NDC = D // P         # 8 contraction chunks for projections
NKT = T // P         # 16 key tiles
NFT = FH // P        # 4 feature tiles per core

f32 = mybir.dt.float32
r32 = mybir.dt.float32r
i16 = mybir.dt.int16
SCALE = 0.125  # 1/sqrt(Hd), applied at the exp stage
ADD = mybir.AluOpType.add
MULT = mybir.AluOpType.mult
EXP = mybir.ActivationFunctionType.Exp

# Schraudolph fast-exp in bf16: bf16 bits = top 16 of f32 bits, so
# int16(x*A/2^16 + B/2^16) bit-viewed as bf16 ~= exp(x).  The constant
# multiplicative bias of the approximation cancels in softmax num/den; only
# the ~2% rms sawtooth survives, attenuated by 1/sqrt(N_eff) in the output.
EXP_A = 12102203.161561485 / 65536.0 * SCALE   # 2**23/ln2/2**16 * scale
EXP_B = 1064866805.0 / 65536.0
# c-chunks whose exp runs on DVE (Schraudolph) instead of ACT
DVE_EXP_SET = (1, 4, 6, 9, 11, 14)

_cache = {}


def _round_fp32r(a):
    """Round fp32 array to fp32r (11 mantissa bits, round-nearest-even)."""
    b = np.ascontiguousarray(a, dtype=np.float32).view(np.uint32).astype(np.uint64)
    drop = 12
    half = np.uint64(1 << (drop - 1))
    lsb = (b >> np.uint64(drop)) & np.uint64(1)
    keepmask = np.uint64(~((1 << drop) - 1) & 0xFFFFFFFF)
    r = (b + half - np.uint64(1) + lsb) & keepmask
    return r.astype(np.uint32).view(np.float32).reshape(np.shape(a))


def _build():
    nc = bacc.Bacc(None, target_bir_lowering=False)
    bf16 = mybir.dt.bfloat16
    # packed layouts: per-partition lines are long and DRAM-contiguous
    xh0 = nc.declare_dram_parameter("xh0", [P, NDC * 1024], bf16, isOutput=False)
    xh1 = nc.declare_dram_parameter("xh1", [P, NDC * 1024], bf16, isOutput=False)
    wq = nc.declare_dram_parameter("wq", [P, NDC * FH], bf16, isOutput=False)
    wk = nc.declare_dram_parameter("wk", [P, NDC * FH], bf16, isOutput=False)
    wv = nc.declare_dram_parameter("wv", [P, NDC * FH], bf16, isOutput=False)
    wo = nc.declare_dram_parameter("wo", [P, NFT * D], bf16, isOutput=False)
    bq = nc.declare_dram_parameter("bq", [FH], f32, isOutput=False)
    bk = nc.declare_dram_parameter("bk", [FH], f32, isOutput=False)
    bvr = nc.declare_dram_parameter("bvr", [P, FH], f32, isOutput=False)
    keep = nc.declare_dram_parameter("keep", [T], r32, isOutput=False)
    outT = nc.declare_dram_parameter("outT", [D, T], f32, isOutput=True)
    xh = [xh0, xh1]

    with tile.TileContext(nc) as tc, ExitStack() as ctx:
        const = ctx.enter_context(tc.tile_pool(name="const", bufs=1))
        qt_pool = ctx.enter_context(tc.tile_pool(name="qt", bufs=1))
        kt_pool = ctx.enter_context(tc.tile_pool(name="kt", bufs=1))
        v_pool = ctx.enter_context(tc.tile_pool(name="v", bufs=1))
        o_pool = ctx.enter_context(tc.tile_pool(name="o", bufs=1))
        ps = ctx.enter_context(tc.tile_pool(name="ps", bufs=1, space="PSUM"))
        x1_pool = ctx.enter_context(tc.tile_pool(name="x1", bufs=2))
        w_pool = ctx.enter_context(tc.tile_pool(name="w", bufs=1))

        # engine warmups: pay ACT exp-table load, custom-DVE table config,
        # and GPSIMD launch costs during pass 0 instead of attention start
        warm = const.tile([1, 16], f32, tag="warm")
        nc.vector.memset(warm, 1.0)
        warm2 = const.tile([1, 16], f32, tag="warm2")
        nc.scalar.activation(warm2, warm, EXP)
        warm3 = const.tile([1, 16], f32, tag="warm3")
        nc.vector.reciprocal_approx_fast(warm3, warm)
        warmb = const.tile([Hd, 16], f32, tag="warmb")
        nc.gpsimd.partition_broadcast(warmb, warm)

        # constants / biases
        bq_sb = const.tile([P, NFT], f32, tag="bq")
        bk_sb = const.tile([P, NFT], f32, tag="bk")
        nc.sync.dma_start(out=bq_sb, in_=bq.rearrange("(f p) -> p f", p=P))
        nc.sync.dma_start(out=bk_sb, in_=bk.rearrange("(f p) -> p f", p=P))
        keep_sb = const.tile([P, NKT], r32, tag="keep")
        nc.sync.dma_start(out=keep_sb, in_=keep.rearrange("(c p) -> p c", p=P))
        zeros8 = const.tile([P, HH], f32, tag="zeros8")
        nc.vector.memset(zeros8, 0.0)

        # persistent activations (bf16: fp8 is too noisy for the 2e-2
        # tolerance -- softmax weight noise passes straight to rel err)
        QT = [qt_pool.tile([P, T], bf16, tag=f"qt{i}", name=f"qt{i}")
              for i in range(NFT)]
        KT = [kt_pool.tile([P, T], bf16, tag=f"kt{i}", name=f"kt{i}")
              for i in range(NFT)]
        V = [v_pool.tile([P, HH, Hd + 1], bf16, tag=f"v{i}", name=f"v{i}")
             for i in range(NKT)]
        O = [o_pool.tile([P, T], bf16, tag=f"o{i}", name=f"o{i}")
             for i in range(NFT)]

        # Q/K weights: one packed tile each, [128, dc, f]; DMAs are issued
        # inside pass 0, interleaved with the x chunks
        wq_b = w_pool.tile([P, NDC, FH], bf16, tag="wqb", name="wq_b")
        wk_b = w_pool.tile([P, NDC, FH], bf16, tag="wkb", name="wk_b")

        def psum_wide(name):
            return ps.tile([P, 1024], f32, tag="st", bufs=2, name=name)

        def psum_pv(name, which):
            return ps.tile([P, 512], f32, tag=("pva" if which == 0 else "pvb"),
                           bufs=2, name=name)

        def qk_psum(f, n, xb, off):
            # xb: [P, dc, 1024] packed half tile; off: column offset in half
            ts = slice(n * 512, (n + 1) * 512)
            fs = slice(f * P, (f + 1) * P)
            pqk = psum_wide("pqk")
            psq = pqk[:, 0:512]
            for dc in range(NDC):
                nc.tensor.matmul(psq, wq_b[:, dc, fs],
                                 xb[:, dc, off:off + 512],
                                 start=(dc == 0), stop=(dc == NDC - 1))
            nc.vector.tensor_scalar_add(
                QT[f][:, ts], psq, bq_sb[:, f:f + 1])
            psk = pqk[:, 512:1024]
            for dc in range(NDC):
                nc.tensor.matmul(psk, wk_b[:, dc, fs],
                                 xb[:, dc, off:off + 512],
                                 start=(dc == 0), stop=(dc == NDC - 1))
            nc.vector.tensor_scalar_add(
                KT[f][:, ts], psk, bk_sb[:, f:f + 1])

        # ------- pass 0: V projection + Q/K projection ----------------
        with nc.named_scope("v_qk0"), ExitStack() as p0:
            wv_pool = p0.enter_context(tc.tile_pool(name="wv", bufs=1))
            vt_pool = p0.enter_context(tc.tile_pool(name="vt", bufs=2))
            xbs = [x1_pool.tile([P, NDC, 1024], bf16, tag="xh", name="xb")
                   for _ in range(2)]
            for i in range(4):
                cs2 = slice(i * 2 * FH, (i + 1) * 2 * FH)
                for k in range(2):
                    dc = 2 * i + k
                    nc.sync.dma_start(
                        out=xbs[0][:, dc, :],
                        in_=xh[0][:, dc * 1024:(dc + 1) * 1024])
                nc.sync.dma_start(out=wq_b[:, 2 * i:2 * i + 2, :],
                                  in_=wq[:, cs2])
                nc.sync.dma_start(out=wk_b[:, 2 * i:2 * i + 2, :],
                                  in_=wk[:, cs2])
            bvr_sb = vt_pool.tile([P, FH], f32, tag="bvr", bufs=1,
                                  name="bvr_sb")
            nc.sync.dma_start(out=bvr_sb, in_=bvr[:])
            wv_b = wv_pool.tile([P, NDC, FH], bf16, tag="wvb", name="wv_b")
            for i in range(4):
                cs2 = slice(i * 2 * FH, (i + 1) * 2 * FH)
                nc.sync.dma_start(out=wv_b[:, 2 * i:2 * i + 2, :],
                                  in_=wv[:, cs2])
            for dc in range(NDC):
                nc.sync.dma_start(
                    out=xbs[1][:, dc, :],
                    in_=xh[1][:, dc * 1024:(dc + 1) * 1024])
            for nh in range(2):
                xb = xbs[nh]
                for f_ in range(2):
                    for nn in range(2):
                        qk_psum(f_, nh * 2 + nn, xb, nn * 512)
                for s in range(8):
                    tidx = nh * 8 + s
                    ss = slice(s * P, (s + 1) * P)
                    psv = psum_pv("psv", s % 2)
                    for dc in range(NDC):
                        nc.tensor.matmul(psv, xb[:, dc, ss],
                                         wv_b[:, dc, :],
                                         start=(dc == 0),
                                         stop=(dc == NDC - 1))
                    vtmp = vt_pool.tile([P, FH], f32, tag="vtmp",
                                        name="vtmp")
                    nc.vector.tensor_tensor(vtmp, psv, bvr_sb,
                                            op=ADD)
                    nc.vector.tensor_scalar_mul(
                        V[tidx][:, :, 0:Hd],
                        vtmp.rearrange("p (h d) -> p h d", h=HH),
                        keep_sb[:, tidx:tidx + 1].bitcast(f32))
                    nc.vector.tensor_scalar_add(
                        V[tidx][:, :, Hd], zeros8,
                        keep_sb[:, tidx:tidx + 1].bitcast(f32))

        # ------- attention + deferred Q/K tiles + projection ----------
        with ExitStack() as pw:
            pt_pool = pw.enter_context(tc.tile_pool(name="pt", bufs=6))
            rc_pool = pw.enter_context(tc.tile_pool(name="rc", bufs=2))
            wo_pool = pw.enter_context(tc.tile_pool(name="wo", bufs=1))
            ot_pool = pw.enter_context(tc.tile_pool(name="ot", bufs=6))

            wo_b = wo_pool.tile([P, NFT, D], bf16, tag="wob", name="wo_b")
            for i in range(2):
                nc.sync.dma_start(out=wo_b[:, 2 * i:2 * i + 2, :],
                                  in_=wo[:, i * 2 * D:(i + 1) * 2 * D])

            alloc_cnt = {0: 0, 1: 0}
            live_slot = {0: -1, 1: -1}

            def pv_alloc(which):
                t = psum_pv("pv", which)
                live_slot[which] = alloc_cnt[which] % 2
                alloc_cnt[which] += 1
                return t

            def pso_alloc(which):
                if alloc_cnt[which] % 2 == live_slot[which]:
                    psum_pv("dummy", which)   # parity shift, no readers
                    alloc_cnt[which] += 1
                t = psum_pv("pso", which)
                alloc_cnt[which] += 1
                return t

            def proj_piece(j, dt_):
                js = slice(j * 512, (j + 1) * 512)
                ds_ = slice(dt_ * P, (dt_ + 1) * P)
                pso = pso_alloc(dt_ % 2)
                for fc in range(NFT):
                    nc.tensor.matmul(pso,
                                     wo_b[:, fc, ds_],
                                     O[fc][:, js],
                                     start=(fc == 0),
                                     stop=(fc == NFT - 1))
                ot = ot_pool.tile([P, 512], f32, tag="ot", name="ot")
                nc.scalar.copy(ot, pso)
                nc.sync.dma_start(out=outT[ds_, js], in_=ot)

            def proj_j(j):
                for dt_ in range(NDC):
                    proj_piece(j, dt_)

            def normalize(hp, j, pvA, pvB):
                js = slice(j * 512, (j + 1) * 512)
                for h, pv in ((0, pvA), (1, pvB)):
                    den = rc_pool.tile([1, 512], f32, tag="den",
                                       bufs=2, name="den")
                    nc.vector.tensor_copy(den, pv[Hd:Hd + 1, :])
                    rec = rc_pool.tile([1, 512], f32, tag="rec",
                                       bufs=2, name="rec")
                    nc.vector.reciprocal_approx_fast(rec, den)
                    rrep = rc_pool.tile([Hd, 512], f32, tag="rrep",
                                        bufs=2, name="rrep")
                    nc.gpsimd.partition_broadcast(rrep, rec)
                    rows = slice(h * Hd, (h + 1) * Hd)
                    nc.vector.tensor_tensor(
                        O[hp][rows, js], pv[0:Hd, :], rrep, op=MULT)

            NT = NFT * NCHUNK * NKT
            sts = {}
            pvs = {}

            def idx(t):
                j, r = divmod(t, NFT * NKT)
                hp, c = divmod(r, NKT)
                return hp, j, c

            def scores(t):
                hp, j, c = idx(t)
                js = slice(j * 512, (j + 1) * 512)
                cs = slice(c * P, (c + 1) * P)
                st = psum_wide("st")
                nc.tensor.matmul(st[:, 0:512],
                                 KT[hp][0:64, cs],
                                 QT[hp][0:64, js],
                                 start=True, stop=True,
                                 tile_position=(0, 0))
                nc.tensor.matmul(st[:, 512:1024],
                                 KT[hp][64:128, cs],
                                 QT[hp][64:128, js],
                                 start=True, stop=True,
                                 tile_position=(64, 0))
                sts[t] = st

            def norm_head(hp, j, h, pv):
                js = slice(j * 512, (j + 1) * 512)
                den = rc_pool.tile([1, 512], f32, tag="den",
                                   bufs=2, name="den")
                nc.vector.tensor_copy(den, pv[Hd:Hd + 1, :])
                rec = rc_pool.tile([1, 512], f32, tag="rec",
                                   bufs=2, name="rec")
                nc.vector.reciprocal_approx_fast(rec, den)
                rrep = rc_pool.tile([Hd, 512], f32, tag="rrep",
                                    bufs=2, name="rrep")
                nc.gpsimd.partition_broadcast(rrep, rec)
                rows = slice(h * Hd, (h + 1) * Hd)
                nc.vector.tensor_tensor(
                    O[hp][rows, js], pv[0:Hd, :], rrep, op=MULT)

            def qk_piece(f, n):
                # deferred Q/K projection for one (f, T-chunk): two
                # parity-guarded [P,512] psum pieces on the pv tags
                ts = slice(n * 512, (n + 1) * 512)
                fs = slice(f * P, (f + 1) * P)
                xb = xbs[n // 2]
                off = (n % 2) * 512
                for which, wb, dst, bias in ((0, wq_b, QT, bq_sb),
                                             (1, wk_b, KT, bk_sb)):
                    pp = pso_alloc(which)
                    for dc in range(NDC):
                        nc.tensor.matmul(pp, wb[:, dc, fs],
                                         xb[:, dc, off:off + 512],
                                         start=(dc == 0),
                                         stop=(dc == NDC - 1))
                    nc.vector.tensor_scalar_add(
                        dst[f][:, ts], pp, bias[:, f:f + 1])

            def attn():
                scores(0)
                scores(1)
                pending = []
                proj_pieces = []
                qk_deferred = [(f, n) for f in (2, 3) for n in range(4)]
                for t in range(NT):
                    hp, j, c = idx(t)
                    js = slice(j * 512, (j + 1) * 512)
                    if c == 0:
                        pvs[0] = (pv_alloc(0), pv_alloc(1))
                    pvA, pvB = pvs[0]
                    if t + 2 < NT:
                        scores(t + 2)
                    st = sts.pop(t)
                    pt = pt_pool.tile([P, 1024], bf16, tag="pt",
                                      name="pt")
                    if c in DVE_EXP_SET:
                        nc.vector.tensor_scalar(
                            out=pt.bitcast(i16), in0=st,
                            scalar1=EXP_A, scalar2=EXP_B,
                            op0=MULT, op1=ADD)
                    else:
                        nc.scalar.activation(pt, st, EXP, scale=SCALE)
                    if c == 2 and pending:
                        php, pj, ppvA, ppvB = pending[0]
                        norm_head(php, pj, 0, ppvA)
                    if c == 5 and pending:
                        php, pj, ppvA, ppvB = pending.pop(0)
                        norm_head(php, pj, 1, ppvB)
                        if php == NFT - 1:
                            proj_pieces.extend((pj, d) for d in range(NDC))
                    if c == 6:
                        for _ in range(2):
                            if proj_pieces:
                                pj2, d2 = proj_pieces.pop(0)
                                proj_piece(pj2, d2)
                    if c in (7, 10, 13) and qk_deferred:
                        qk_piece(*qk_deferred.pop(0))
                    nc.tensor.matmul(pvA[0:Hd + 1, :],
                                     V[c][:, 2 * hp, :],
                                     pt[:, 0:512],
                                     start=(c == 0),
                                     stop=(c == NKT - 1))
                    nc.tensor.matmul(pvB[0:Hd + 1, :],
                                     V[c][:, 2 * hp + 1, :],
                                     pt[:, 512:1024],
                                     start=(c == 0),
                                     stop=(c == NKT - 1))
                    if c == NKT - 1:
                        pending.append((hp, j, pvA, pvB))
                for php, pj, ppvA, ppvB in pending:
                    norm_head(php, pj, 0, ppvA)
                    norm_head(php, pj, 1, ppvB)
                    if php == NFT - 1:
                        proj_pieces.extend((pj, d) for d in range(NDC))
                pending.clear()
                for pj2, d2 in proj_pieces:
                    proj_piece(pj2, d2)
                proj_pieces.clear()

            with nc.named_scope("attn"):
                attn()

    nc.compile()
    return nc


def _get_nc():
    if "nc" not in _cache:
        _cache["nc"] = _build()
    return _cache["nc"]


def kernel(x, mask, Wq, bq, Wk, bk, Wv, bv, Wo, bo):
    x = np.asarray(x, dtype=np.float32)
    mask = np.asarray(mask)
    Wq = np.asarray(Wq, dtype=np.float32)
    bq = np.asarray(bq, dtype=np.float32)
    Wk = np.asarray(Wk, dtype=np.float32)
    bk = np.asarray(bk, dtype=np.float32)
    Wv = np.asarray(Wv, dtype=np.float32)
    bv = np.asarray(bv, dtype=np.float32)
    Wo = np.asarray(Wo, dtype=np.float32)
    bo = np.asarray(bo, dtype=np.float32)

    scale = np.float32(Hd) ** -0.5
    nc = _get_nc()

    def pack_w(w):
        # [D, FH] -> [128, (dc f)]: partition p line = concat over dc of
        # w[dc*128+p, :]
        return np.ascontiguousarray(
            w.astype(ml_dtypes.bfloat16).reshape(NDC, P, FH)
            .transpose(1, 0, 2).reshape(P, NDC * FH))

    in_maps = []
    for core in range(8):
        b, s = core // 2, core % 2
        sl = slice(s * FH, (s + 1) * FH)
        xr = x[b].T.astype(ml_dtypes.bfloat16).reshape(NDC, P, T)
        wo_p = (Wo[sl, :].astype(ml_dtypes.bfloat16)
                .reshape(NFT, P, D).transpose(1, 0, 2).reshape(P, NFT * D))
        m = {
            "xh0": np.ascontiguousarray(
                xr[:, :, 0:1024].transpose(1, 0, 2).reshape(P, NDC * 1024)),
            "xh1": np.ascontiguousarray(
                xr[:, :, 1024:2048].transpose(1, 0, 2).reshape(P, NDC * 1024)),
            "wq": pack_w(Wq[:, sl]),
            "wk": pack_w(Wk[:, sl]),
            "wv": pack_w(Wv[:, sl]),
            "wo": np.ascontiguousarray(wo_p),
            "bq": np.ascontiguousarray(bq[sl]),
            "bk": np.ascontiguousarray(bk[sl]),
            "bvr": np.ascontiguousarray(np.broadcast_to(bv[sl], (P, FH))),
            "keep": (1.0 - mask[b].astype(np.float32)),
        }
        in_maps.append(m)

    global _last_in_maps
    _last_in_maps = in_maps
    res = run_bass_kernel_spmd(nc, in_maps, list(range(8)))
    out = np.empty((B, T, D), dtype=np.float32)
    for b in range(B):
        acc = res.results[2 * b]["outT"] + res.results[2 * b + 1]["outT"]
        out[b] = acc.T + bo
    return out

